# revision 27
# baseline (speedup 1.0000x reference)
"""Bahdanau additive attention on TRN2 — separable-Fourier Bass/Tile kernel, v2.2.

Problem: nn_AttentionLayer_11055245820581
  e[b,y,x] = softmax_x( sum_e V[e] * tanh(Ws[b,x,e] + Uh[b,y,e]) )
  c[b,y,:] = sum_x e[b,y,x] * enc[b,x,:]
with Ws = enc @ W_a, Uh = dec @ U_a.

Sharding: data-parallel over batch B=8 across the 8 NeuronCores.

tanh(z) ~= sum_{m=1..M} c_m sin(m*w*z) on |z| <= 7.7 (w = pi/7.7), and
sin(mw(a+b)) = sin_m(a)cos_m(b) + cos_m(a)sin_m(b) turns the V-weighted
e-contraction into 2M rank-E fp16 PE matmuls.

Factor families advance by STRIDE-2 dual Chebyshev chains with the
multiplier t2c2 = 2cos(2wz) (from ONE ACT Square + tensor_scalar):
  odd : fam3 = (t2c2 +- 1).fam1, fam5 = t2c2.fam3 - fam1,
        fam7 = t2c2.fam5 - fam3
  even: fam2' (= fam2/2) from s1*c1 / Square(c1),
        fam4' = t2c2.fam2' - fam0', fam6' = t2c2.fam4' - fam2'
(halved even chain folds into gamma = 4c_m for even m). fam4/5 and
fam6/7 mults are merged [128,4096] DVE TTs; fam7 is finished before fam6
(famUs7 at tc.high_priority) so the mode-7 matmuls overlap the fam6
subtract.  Per-mode c_m scalings: modes 1..M-2 on ACT Copy-with-scale
(idle mid-chain), last two on DVE tensor_scalar @4x.  UhT/WsT live in
ONE adjacent PSUM tile so each trig base is a single merged [128,1024]
ACT Sin over both sides.  Mode-2 sin bases come from one DVE TT (s1*c1).
U16 cast on idle prologue DVE (gpsimd tensor ops stall DVE).  Last-mode
matmuls emitted xh-major with EXP split per xh half; PE keepalive
matmuls hold the clock before the tail burst.  enc DMA is issued on
gpsimd BEFORE the identity build so encT transposes start earlier.

HW-measured pitfalls baked in (TRN2):
 - multi-free-dim DVE APs: only t2bcast-class shapes (0-stride lead dims,
   contiguous 512+ inner runs); everything else flat.
 - dma_start_transpose executes ON the issuing sequencer (~1us per
   [128,128] block) — NOT used; PE identity transposes instead.
 - explicit start/stop accumulation bits across interleaved PSUM groups
   misassociate: memset once + start=False/stop=False everywhere.
 - Sin and Exp live in different ACT table sets; the swap is placed
   mid-chain via a dummy Exp pinned on famUs3.
"""

import os

os.environ.setdefault("NEURON_RT_RESET_CORES", "1")

import numpy as np
from contextlib import ExitStack

import concourse.bass as bass
import concourse.bacc as bacc
import concourse.tile as tile
from concourse import mybir
from concourse.bass_utils import run_bass_kernel_spmd

B, Tx, Ty, E, D = 8, 256, 256, 256, 256
P = 128
NCORES = 8
F32 = mybir.dt.float32
F16 = mybir.dt.float16
SIN = mybir.ActivationFunctionType.Sin
EXP = mybir.ActivationFunctionType.Exp
SQUARE = mybir.ActivationFunctionType.Square
IDENT = mybir.ActivationFunctionType.Identity
COPY = mybir.ActivationFunctionType.Copy
MULT = mybir.AluOpType.mult
ADD = mybir.AluOpType.add
SUB = mybir.AluOpType.subtract

EC = E // P
XC = Tx // P
YC = Ty // P
DC = D // P

M_MODES = 6
L_PER = 7.7
OMEGA = float(np.pi / L_PER)
# M=7 fit (wstd=3.2 floor=5e-3): end-to-end 4.2e-3
COEF7 = [1.227222613856828, -0.06362063635995319, 0.3190074912395547,
         -0.07340173334525109, 0.12816602876155422, -0.043337027089728836,
         0.039055147705349964]
# M=6 fit (wstd=1.6 floor=1e-2): end-to-end 1.13e-2
COEF6 = [1.2507316474740167, -0.0931110861002801, 0.3208150966954934,
         -0.018198029982230808, 0.026491774049749867, 0.04957783284348268]
COEF = COEF7 if M_MODES == 7 else COEF6

_NC = None
LAST_RESULTS = None


def _mk(t, off, dims):
    return bass.AP(tensor=t.tensor, offset=t.offset + off,
                   ap=[t.ap[0]] + [[s, c] for (s, c) in dims])


def _build_body(tc, ctx, enc_d, dec_d, W_d, U_d, V_d, c_d, e_d):
    nc = tc.nc
    from concourse.masks import make_identity

    consts = ctx.enter_context(tc.tile_pool(name="consts", bufs=1))
    psA = ctx.enter_context(tc.tile_pool(name="psA", bufs=1, space="PSUM"))
    pieces = ctx.enter_context(tc.tile_pool(name="pieces", bufs=4,
                                            space="PSUM"))

    # ---- staging tiles ----
    dec_sb = consts.tile([P, YC, D], F32)
    enc_sb = consts.tile([P, XC, E], F32)
    U_sb = consts.tile([P, DC, E], F32)
    W_sb = consts.tile([P, EC, E], F32)
    V_sb = consts.tile([P, EC], F32)

    # ---- input DMA ----
    # dec gates transposes -> UhT -> the trig bases (the whole head):
    # quarter-split across BOTH hwdge rings, issued first.
    # sync carries 3 dec quarters; scalar takes one quarter then U so U
    # lands in time for the UhT matmuls (it was gating them at +1.3us)
    qs = [(0, 0, nc.sync), (0, 1, nc.scalar), (1, 0, nc.sync),
          (1, 1, nc.sync)]
    for yh, dh, eng in qs:
        eng.dma_start(
            out=dec_sb[:, yh, dh * P:(dh + 1) * P],
            in_=dec_d[yh * P:(yh + 1) * P, dh * P:(dh + 1) * P])
    nc.scalar.dma_start(out=U_sb[:],
                        in_=U_d.rearrange("(c p) e -> p c e", c=DC))
    for xh in range(XC):
        nc.sync.dma_start(out=enc_sb[:, xh, :],
                          in_=enc_d[xh * P:(xh + 1) * P, :])
    nc.scalar.dma_start(out=W_sb[:],
                        in_=W_d.rearrange("(c p) e -> p c e", c=EC))
    nc.sync.dma_start(out=V_sb[:],
                      in_=V_d.rearrange("(c p) o -> p (c o)", c=EC))

    # identity for PE transposes; fp16 copy late (only for the epilogue)
    ident = consts.tile([P, P], F32)
    make_identity(nc, ident)
    ident16 = consts.tile([P, P], F16)
    nc.gpsimd.tensor_copy(ident16[:], ident[:])

    # ---- warmups ----
    halfpi_sb = consts.tile([P, 1], F32)
    nc.vector.memset(halfpi_sb[:], float(np.pi / 2))
    warm_sb = consts.tile([P, 1], F32)
    nc.scalar.activation(out=warm_sb[:], in_=halfpi_sb[:], func=SIN,
                         scale=0.1)
    pe_warm = consts.tile([P, 256], F16)
    nc.vector.memset(pe_warm[:], 1.0)
    for r in range(4):
        warm_ps = pieces.tile([P, 512], F32, tag="piece", name=f"warm{r}")
        nc.tensor.matmul(out=warm_ps[:, :256], lhsT=pe_warm[:, :P],
                         rhs=pe_warm[:], start=True, stop=True,
                         skip_group_check=True)

    ones16 = consts.tile([P, 1], F16)
    nc.vector.memset(ones16[:], 1.0)

    logit_ps = psA.tile([P, XC, Ty], F32)
    nc.vector.memset(logit_ps[:], 0.0)
    psAB = psA.tile([P, 2, EC, 256], F32)   # [e-part, side(0=U,1=W), co, y/x]

    # ---- fp16 casts ----
    U16 = consts.tile([P, DC, E], F16)
    W16 = consts.tile([P, EC, E], F16)
    enc16 = consts.tile([P, XC, E], F16)
    nc.vector.tensor_copy(_mk(U16, 0, [(1, 512)]), _mk(U_sb, 0, [(1, 512)]))
    nc.scalar.copy(_mk(W16, 0, [(1, 512)]), _mk(W_sb, 0, [(1, 512)]))

    V2_sb = consts.tile([P, EC], F32)
    nc.vector.tensor_scalar_mul(out=V2_sb[:], in0=V_sb[:], scalar1=0.5)

    # ---- fp32 PE transposes; evacs cast to fp16 on DVE ----
    decT16 = consts.tile([P, DC, Ty], F16)
    encT16 = consts.tile([P, EC, Tx], F16)
    for i in range(YC):
        for j in range(DC):
            pt = pieces.tile([P, 512], F32, tag="piece", name=f"ptD{i}{j}")
            nc.tensor.transpose(out=pt[:, :P],
                                in_=dec_sb[:, i, j * P:(j + 1) * P],
                                identity=ident[:])
            nc.vector.tensor_copy(decT16[:, j, i * P:(i + 1) * P], pt[:, :P])
    for i in range(XC):
        for j in range(EC):
            pt = pieces.tile([P, 512], F32, tag="piece", name=f"ptE{i}{j}")
            nc.tensor.transpose(out=pt[:, :P],
                                in_=enc_sb[:, i, j * P:(j + 1) * P],
                                identity=ident[:])
            nc.vector.tensor_copy(encT16[:, j, i * P:(i + 1) * P], pt[:, :P])

    # ---- UhT / WsT ----
    for co in range(EC):
        for ci in range(DC):
            nc.tensor.matmul(
                out=psAB[:, 0, co, :],
                lhsT=U16[:, ci, co * P:(co + 1) * P],
                rhs=decT16[:, ci, :],
                start=(ci == 0), stop=(ci == DC - 1))
    for co in range(EC):
        for ci in range(EC):
            nc.tensor.matmul(
                out=psAB[:, 1, co, :],
                lhsT=W16[:, ci, co * P:(co + 1) * P],
                rhs=encT16[:, ci, :],
                start=(ci == 0), stop=(ci == EC - 1))

    # ---- factor tiles ----
    fam1 = consts.tile([P, 3072], F16)
    famA = consts.tile([P, 4096], F16)   # [fam2' | fam3]
    famB = consts.tile([P, 4096], F16)   # [fam4' | fam5]
    famC = consts.tile([P, 4096], F16)   # [fam6' | fam7]
    t2sq = consts.tile([P, 1024], F16)
    t2c2 = consts.tile([P, 1024], F16)
    m3m = consts.tile([P, 2048], F16)
    famUs = [None] * (M_MODES + 1)
    for m in range(1, M_MODES + 1):
        famUs[m] = consts.tile([P, 1024], F16, name=f"famUs{m}")

    # ---- trig bases, per side: the U pair starts as soon as UhT is done
    # instead of waiting for WsT ----
    psU = _mk(psAB, 0, [(1, 512)])
    psW = _mk(psAB, 512, [(1, 512)])
    nc.scalar.activation(out=_mk(fam1, 0, [(1, 512)]),
                         in_=psU, func=SIN, scale=OMEGA)
    nc.scalar.activation(out=_mk(fam1, 1024, [(1, 512)]),
                         in_=psU, func=SIN, scale=-OMEGA, bias=halfpi_sb[:])
    nc.scalar.activation(out=_mk(fam1, 2048, [(1, 512)]),
                         in_=psW, func=SIN, scale=OMEGA)
    nc.scalar.activation(out=_mk(fam1, 2560, [(1, 512)]),
                         in_=psW, func=SIN, scale=-OMEGA, bias=halfpi_sb[:])
    nc.scalar.activation(out=_mk(t2sq, 0, [(1, 512)]),
                         in_=_mk(fam1, 1024, [(1, 512)]), func=SQUARE)
    nc.scalar.activation(out=_mk(t2sq, 512, [(1, 512)]),
                         in_=_mk(fam1, 2560, [(1, 512)]), func=SQUARE)

    def famUs_act(m, src, base, gamma):
        nc.scalar.activation(
            out=_mk(famUs[m], 0, [(512, 2), (1, 512)]),
            in_=_mk(src, base, [(1024, 2), (1, 512)]),
            func=COPY, scale=float(gamma))

    def famUs_dve(m, src, base, gamma):
        nc.vector.tensor_scalar_mul(
            out=_mk(famUs[m], 0, [(512, 2), (1, 512)]),
            in0=_mk(src, base, [(1024, 2), (1, 512)]),
            scalar1=float(gamma))

    # ---- DVE factor chain ----
    for ec in range(EC):
        nc.vector.tensor_scalar_mul(
            out=_mk(fam1, 512 + ec * 256, [(1, 256)]),
            in0=_mk(fam1, 2048 + ec * 256, [(1, 256)]),
            scalar1=V_sb[:, ec:ec + 1])
    for ec in range(EC):
        nc.vector.tensor_scalar_mul(
            out=_mk(fam1, 1536 + ec * 256, [(1, 256)]),
            in0=_mk(fam1, 2560 + ec * 256, [(1, 256)]),
            scalar1=V_sb[:, ec:ec + 1])
    famUs_act(1, fam1, 0, COEF[0])
    nc.vector.tensor_tensor(
        out=_mk(famA, 0, [(512, 2), (1, 512)]),
        in0=_mk(fam1, 0, [(2048, 2), (1, 512)]),
        in1=_mk(fam1, 1024, [(1536, 2), (1, 512)]), op=MULT)
    for ec in range(EC):
        nc.vector.tensor_scalar_mul(
            out=_mk(famA, 512 + ec * 256, [(1, 256)]),
            in0=_mk(famA, 512 + ec * 256, [(1, 256)]),
            scalar1=V_sb[:, ec:ec + 1])
    nc.vector.tensor_scalar(out=t2c2[:], in0=t2sq[:], scalar1=4.0,
                            scalar2=2.0, op0=MULT, op1=SUB)
    nc.vector.tensor_scalar(out=_mk(m3m, 0, [(1, 1024)]), in0=t2sq[:],
                            scalar1=4.0, scalar2=1.0, op0=MULT, op1=SUB)
    nc.vector.tensor_scalar(out=_mk(m3m, 1024, [(1, 1024)]), in0=t2sq[:],
                            scalar1=4.0, scalar2=3.0, op0=MULT, op1=SUB)
    nc.vector.tensor_scalar(out=_mk(famA, 1024, [(1, 512)]),
                            in0=_mk(t2sq, 0, [(1, 512)]),
                            scalar1=0.5, scalar2=None, op0=SUB)
    for ec in range(EC):
        nc.vector.tensor_scalar(
            out=_mk(famA, 1536 + ec * 256, [(1, 256)]),
            in0=_mk(t2sq, 512 + ec * 256, [(1, 256)]),
            scalar1=0.5, scalar2=V_sb[:, ec:ec + 1], op0=SUB, op1=MULT)
    famUs_act(2, famA, 0, 4.0 * COEF[1])
    nc.vector.tensor_tensor(out=_mk(famA, 2048, [(1, 2048)]),
                            in0=_mk(fam1, 0, [(1, 2048)]),
                            in1=_mk(m3m, 0, [(1, 2048)]), op=MULT)
    famUs_act(3, famA, 2048, COEF[2])
    nc.scalar.activation(out=warm_sb[:], in_=_mk(famUs[3], 0, [(1, 1)]),
                         func=EXP)
    t2c2_b4 = _mk(t2c2, 0, [(0, 2), (0, 2), (1, 1024)])
    sh4096 = [(2048, 2), (1024, 2), (1, 1024)]
    nc.vector.tensor_tensor(out=_mk(famB, 0, sh4096),
                            in0=_mk(famA, 0, sh4096),
                            in1=t2c2_b4, op=MULT)
    nc.vector.tensor_scalar(out=_mk(famB, 1024, [(1, 512)]),
                            in0=_mk(famB, 1024, [(1, 512)]),
                            scalar1=0.5, scalar2=None, op0=SUB)
    for ec in range(EC):
        nc.vector.tensor_scalar(
            out=_mk(famB, 1536 + ec * 256, [(1, 256)]),
            in0=_mk(famB, 1536 + ec * 256, [(1, 256)]),
            scalar1=V2_sb[:, ec:ec + 1], scalar2=None, op0=SUB)
    famUs_act(4, famB, 0, 4.0 * COEF[3])
    nc.scalar.copy(_mk(enc16, 0, [(1, 512)]),
                   _mk(enc_sb, 0, [(1, 512)]))   # ACT mid-chain slack
    nc.vector.tensor_tensor(out=_mk(famB, 2048, [(1, 2048)]),
                            in0=_mk(famB, 2048, [(1, 2048)]),
                            in1=_mk(fam1, 0, [(1, 2048)]), op=SUB)
    famUs_act(5, famB, 2048, COEF[4])
    if M_MODES >= 7:
        nc.vector.tensor_tensor(out=_mk(famC, 0, sh4096),
                                in0=_mk(famB, 0, sh4096),
                                in1=t2c2_b4, op=MULT)
        nc.vector.tensor_tensor(out=_mk(famC, 2048, [(1, 2048)]),
                                in0=_mk(famC, 2048, [(1, 2048)]),
                                in1=_mk(famA, 2048, [(1, 2048)]), op=SUB)
        with tc.high_priority():
            famUs_dve(7, famC, 2048, COEF[6])
        nc.vector.tensor_tensor(out=_mk(famC, 0, [(1, 2048)]),
                                in0=_mk(famC, 0, [(1, 2048)]),
                                in1=_mk(famA, 0, [(1, 2048)]), op=SUB)
        with tc.high_priority():
            famUs_dve(6, famC, 0, 4.0 * COEF[5])
    else:
        sh2048 = [(1024, 2), (1, 1024)]
        nc.vector.tensor_tensor(out=_mk(famC, 0, sh2048),
                                in0=_mk(famB, 0, sh2048),
                                in1=_mk(t2c2, 0, [(0, 2), (1, 1024)]),
                                op=MULT)
        nc.vector.tensor_tensor(out=_mk(famC, 0, [(1, 2048)]),
                                in0=_mk(famC, 0, [(1, 2048)]),
                                in1=_mk(famA, 0, [(1, 2048)]), op=SUB)
        with tc.high_priority():
            famUs_dve(6, famC, 0, 4.0 * COEF[5])

    # ---- logit matmuls ----
    fam_base = {1: (fam1, 0), 2: (famA, 0), 3: (famA, 2048),
                4: (famB, 0), 5: (famB, 2048), 6: (famC, 0),
                7: (famC, 2048)}

    def emit_mode(m, xhs):
        t, base = fam_base[m]
        for xh in xhs:
            for f in range(2):
                for ec in range(EC):
                    nc.tensor.matmul(
                        out=logit_ps[:, xh, :],
                        lhsT=_mk(t, base + f * 1024 + 512 + ec * 256 + xh * P,
                                 [(1, P)]),
                        rhs=_mk(famUs[m], (1 - f) * 512 + ec * 256,
                                [(1, 256)]),
                        start=False, stop=False, skip_group_check=True)

    last_modes = [M_MODES, M_MODES - 1]
    for m in range(1, M_MODES - 1):
        emit_mode(m, range(XC))
    # PE keepalives: famB sin half after its mult, then famC right after
    # its mult (holds the clock through the fam6/fam7 subtract window; the
    # WAR on famC resolves long before the subs need the buffer)
    for r, src in enumerate((_mk(famB, 512, [(1, P)]),
                             _mk(famB, 768, [(1, P)]),
                             _mk(famUs[M_MODES], 0, [(1, P)]),
                             _mk(famUs[M_MODES], 512, [(1, P)]))):
        ka = pieces.tile([P, 512], F32, tag="piece", name=f"ka{r}")
        nc.tensor.matmul(out=ka[:, :256], lhsT=src,
                         rhs=pe_warm[:], start=True, stop=True,
                         skip_group_check=True)
    expT = consts.tile([P, XC, Ty], F16)
    for m in last_modes:
        emit_mode(m, [0])
    nc.scalar.activation(out=expT[:, 0, :], in_=logit_ps[:, 0, :], func=EXP)
    for m in last_modes:
        emit_mode(m, [1])
    nc.scalar.activation(out=expT[:, 1, :], in_=logit_ps[:, 1, :], func=EXP)

    # ---- softmax epilogue ----
    recip_sb = consts.tile([P, YC], F32)
    alpha = [[consts.tile([P, P], F32, name=f"al{yh}{xh}") for xh in range(XC)]
             for yh in range(YC)]
    c_sb = [consts.tile([P, E], F32, name=f"c{yh}") for yh in range(YC)]
    # den/recip/context for both halves FIRST (keeps the pieces-pool
    # rotation acyclic: the pa transposes below alias these buffers)
    for yh in range(YC):
        den = pieces.tile([P, 512], F32, tag="piece", name=f"den{yh}")
        for xh in range(XC):
            nc.tensor.matmul(out=den[:, :1],
                             lhsT=expT[:, xh, yh * P:(yh + 1) * P],
                             rhs=ones16[:],
                             start=(xh == 0), stop=(xh == XC - 1))
        nc.vector.reciprocal(recip_sb[:, yh:yh + 1], den[:, :1])
        cps = pieces.tile([P, 512], F32, tag="piece", name=f"cps{yh}")
        for xh in range(XC):
            nc.tensor.matmul(out=cps[:, :E],
                             lhsT=expT[:, xh, yh * P:(yh + 1) * P],
                             rhs=enc16[:, xh, :],
                             start=(xh == 0), stop=(xh == XC - 1))
        nc.vector.tensor_scalar_mul(out=c_sb[yh][:], in0=cps[:, :E],
                                    scalar1=recip_sb[:, yh:yh + 1])
        nc.scalar.dma_start(out=c_d[yh * P:(yh + 1) * P, :], in_=c_sb[yh][:])
    for yh in range(YC):
        for xh in range(XC):
            pa = pieces.tile([P, 512], F16, tag="piece", name=f"pa{yh}{xh}")
            nc.tensor.transpose(out=pa[:, :P],
                                in_=expT[:, xh, yh * P:(yh + 1) * P],
                                identity=ident16[:])
            # the LAST scale must not be on ACT: the e-DMA issues on the
            # scalar ring block its sequencer ~1us (recurring trace gap)
            if xh == 0 or yh == YC - 1:
                nc.vector.tensor_scalar_mul(
                    out=alpha[yh][xh][:], in0=pa[:, :P],
                    scalar1=recip_sb[:, yh:yh + 1])
            else:
                nc.scalar.activation(out=alpha[yh][xh][:],
                                     in_=pa[:, :P], func=IDENT,
                                     scale=recip_sb[:, yh:yh + 1])
            eng = nc.sync if xh == 0 else nc.scalar
            eng.dma_start(
                out=e_d[yh * P:(yh + 1) * P, xh * P:(xh + 1) * P],
                in_=alpha[yh][xh][:])


def _build():
    nc = bacc.Bacc("TRN2", target_bir_lowering=False, debug=False,
                   num_devices=NCORES)
    enc_d = nc.dram_tensor("enc", [Tx, E], F32, kind="ExternalInput").ap()
    dec_d = nc.dram_tensor("dec", [Ty, D], F32, kind="ExternalInput").ap()
    W_d = nc.dram_tensor("W", [E, E], F32, kind="ExternalInput").ap()
    U_d = nc.dram_tensor("U", [D, E], F32, kind="ExternalInput").ap()
    V_d = nc.dram_tensor("V", [E, 1], F32, kind="ExternalInput").ap()
    c_d = nc.dram_tensor("c_out", [Ty, E], F32, kind="ExternalOutput").ap()
    e_d = nc.dram_tensor("e_out", [Ty, Tx], F32, kind="ExternalOutput").ap()

    with tile.TileContext(nc) as tc:
        with ExitStack() as ctx:
            _build_body(tc, ctx, enc_d, dec_d, W_d, U_d, V_d, c_d, e_d)
    nc.compile()
    return nc


def _get_nc():
    global _NC
    if _NC is None:
        _NC = _build()
    return _NC


def kernel(encoder_out_seq, decoder_out_seq, W_a, U_a, V_a):
    enc = np.ascontiguousarray(np.asarray(encoder_out_seq, dtype=np.float32))
    dec = np.ascontiguousarray(np.asarray(decoder_out_seq, dtype=np.float32))
    W = np.ascontiguousarray(np.asarray(W_a, dtype=np.float32))
    U = np.ascontiguousarray(np.asarray(U_a, dtype=np.float32))
    V = np.ascontiguousarray(np.asarray(V_a, dtype=np.float32))

    nc = _get_nc()
    in_maps = [
        {"enc": enc[i], "dec": dec[i], "W": W, "U": U, "V": V}
        for i in range(NCORES)
    ]
    res = run_bass_kernel_spmd(nc, in_maps, list(range(NCORES)))
    global LAST_RESULTS
    LAST_RESULTS = res
    c = np.stack([res.results[i]["c_out"] for i in range(NCORES)])
    e = np.stack([res.results[i]["e_out"] for i in range(NCORES)])
    return c, e


# revision 29
# speedup vs baseline: 1.0856x; 1.0856x over previous
"""Bahdanau additive attention on TRN2 — separable-Fourier Bass/Tile kernel, v2.2.

Problem: nn_AttentionLayer_11055245820581
  e[b,y,x] = softmax_x( sum_e V[e] * tanh(Ws[b,x,e] + Uh[b,y,e]) )
  c[b,y,:] = sum_x e[b,y,x] * enc[b,x,:]
with Ws = enc @ W_a, Uh = dec @ U_a.

Sharding: data-parallel over batch B=8 across the 8 NeuronCores.

tanh(z) ~= sum_{m=1..M} c_m sin(m*w*z) on |z| <= 7.7 (w = pi/7.7), and
sin(mw(a+b)) = sin_m(a)cos_m(b) + cos_m(a)sin_m(b) turns the V-weighted
e-contraction into 2M rank-E fp16 PE matmuls.

Factor families advance by STRIDE-2 dual Chebyshev chains with the
multiplier t2c2 = 2cos(2wz) (from ONE ACT Square + tensor_scalar):
  odd : fam3 = (t2c2 +- 1).fam1, fam5 = t2c2.fam3 - fam1,
        fam7 = t2c2.fam5 - fam3
  even: fam2' (= fam2/2) from s1*c1 / Square(c1),
        fam4' = t2c2.fam2' - fam0', fam6' = t2c2.fam4' - fam2'
(halved even chain folds into gamma = 4c_m for even m). fam4/5 and
fam6/7 mults are merged [128,4096] DVE TTs; fam7 is finished before fam6
(famUs7 at tc.high_priority) so the mode-7 matmuls overlap the fam6
subtract.  Per-mode c_m scalings: modes 1..M-2 on ACT Copy-with-scale
(idle mid-chain), last two on DVE tensor_scalar @4x.  UhT/WsT live in
ONE adjacent PSUM tile so each trig base is a single merged [128,1024]
ACT Sin over both sides.  Mode-2 sin bases come from one DVE TT (s1*c1).
U16 cast on idle prologue DVE (gpsimd tensor ops stall DVE).  Last-mode
matmuls emitted xh-major with EXP split per xh half; PE keepalive
matmuls hold the clock before the tail burst.  enc DMA is issued on
gpsimd BEFORE the identity build so encT transposes start earlier.

HW-measured pitfalls baked in (TRN2):
 - multi-free-dim DVE APs: only t2bcast-class shapes (0-stride lead dims,
   contiguous 512+ inner runs); everything else flat.
 - dma_start_transpose executes ON the issuing sequencer (~1us per
   [128,128] block) — NOT used; PE identity transposes instead.
 - explicit start/stop accumulation bits across interleaved PSUM groups
   misassociate: memset once + start=False/stop=False everywhere.
 - Sin and Exp live in different ACT table sets; the swap is placed
   mid-chain via a dummy Exp pinned on famUs3.
"""

import os

os.environ.setdefault("NEURON_RT_RESET_CORES", "1")

import numpy as np
from contextlib import ExitStack

import concourse.bass as bass
import concourse.bacc as bacc
import concourse.tile as tile
from concourse import mybir
from concourse.bass_utils import run_bass_kernel_spmd

B, Tx, Ty, E, D = 8, 256, 256, 256, 256
P = 128
NCORES = 8
F32 = mybir.dt.float32
F16 = mybir.dt.float16
SIN = mybir.ActivationFunctionType.Sin
EXP = mybir.ActivationFunctionType.Exp
SQUARE = mybir.ActivationFunctionType.Square
IDENT = mybir.ActivationFunctionType.Identity
COPY = mybir.ActivationFunctionType.Copy
MULT = mybir.AluOpType.mult
ADD = mybir.AluOpType.add
SUB = mybir.AluOpType.subtract

EC = E // P
XC = Tx // P
YC = Ty // P
DC = D // P

M_MODES = 6
L_PER = 7.7
OMEGA = float(np.pi / L_PER)
# M=7 fit (wstd=3.2 floor=5e-3): end-to-end 4.2e-3
COEF7 = [1.227222613856828, -0.06362063635995319, 0.3190074912395547,
         -0.07340173334525109, 0.12816602876155422, -0.043337027089728836,
         0.039055147705349964]
# M=6 fit (wstd=1.6 floor=1e-2): end-to-end 1.13e-2
COEF6 = [1.2507316474740167, -0.0931110861002801, 0.3208150966954934,
         -0.018198029982230808, 0.026491774049749867, 0.04957783284348268]
COEF = COEF7 if M_MODES == 7 else COEF6

_NC = None
LAST_RESULTS = None


def _mk(t, off, dims):
    return bass.AP(tensor=t.tensor, offset=t.offset + off,
                   ap=[t.ap[0]] + [[s, c] for (s, c) in dims])


def _build_body(tc, ctx, enc_d, dec_d, W_d, U_d, V_d, c_d, e_d):
    nc = tc.nc
    from concourse.masks import make_identity

    consts = ctx.enter_context(tc.tile_pool(name="consts", bufs=1))
    psA = ctx.enter_context(tc.tile_pool(name="psA", bufs=1, space="PSUM"))
    pieces = ctx.enter_context(tc.tile_pool(name="pieces", bufs=4,
                                            space="PSUM"))

    # ---- staging tiles ----
    dec_sb = consts.tile([P, YC, D], F32)
    enc_sb = consts.tile([P, XC, E], F32)
    U_sb = consts.tile([P, DC, E], F32)
    W_sb = consts.tile([P, EC, E], F32)
    V_sb = consts.tile([P, EC], F32)

    # ---- input DMA ----
    # dec gates transposes -> UhT -> the trig bases (the whole head):
    # quarter-split across BOTH hwdge rings, issued first.  U slots
    # BETWEEN the scalar-ring dec quarters: it was landing after both
    # (~12us) and gating the UhT matmuls by ~1.4us.
    def dec_q(yh, dh, eng):
        eng.dma_start(out=dec_sb[:, yh, dh * P:(dh + 1) * P],
                      in_=dec_d[yh * P:(yh + 1) * P, dh * P:(dh + 1) * P])

    dec_q(0, 0, nc.sync)
    dec_q(0, 1, nc.scalar)
    dec_q(1, 0, nc.sync)
    nc.scalar.dma_start(out=U_sb[:],
                        in_=U_d.rearrange("(c p) e -> p c e", c=DC))
    dec_q(1, 1, nc.scalar)
    for xh in range(XC):
        nc.sync.dma_start(out=enc_sb[:, xh, :],
                          in_=enc_d[xh * P:(xh + 1) * P, :])
    nc.scalar.dma_start(out=W_sb[:],
                        in_=W_d.rearrange("(c p) e -> p c e", c=EC))
    nc.sync.dma_start(out=V_sb[:],
                      in_=V_d.rearrange("(c p) o -> p (c o)", c=EC))

    # identity for PE transposes; fp16 copy late (only for the epilogue)
    ident = consts.tile([P, P], F32)
    make_identity(nc, ident)
    ident16 = consts.tile([P, P], F16)
    nc.gpsimd.tensor_copy(ident16[:], ident[:])

    # ---- warmups ----
    halfpi_sb = consts.tile([P, 1], F32)
    nc.vector.memset(halfpi_sb[:], float(np.pi / 2))
    warm_sb = consts.tile([P, 1], F32)
    nc.scalar.activation(out=warm_sb[:], in_=halfpi_sb[:], func=SIN,
                         scale=0.1)
    pe_warm = consts.tile([P, 256], F16)
    nc.vector.memset(pe_warm[:], 1.0)
    for r in range(4):
        warm_ps = pieces.tile([P, 512], F32, tag="piece", name=f"warm{r}")
        nc.tensor.matmul(out=warm_ps[:, :256], lhsT=pe_warm[:, :P],
                         rhs=pe_warm[:], start=True, stop=True,
                         skip_group_check=True)

    ones16 = consts.tile([P, 1], F16)
    nc.vector.memset(ones16[:], 1.0)

    logit_ps = psA.tile([P, XC, Ty], F32)
    nc.vector.memset(logit_ps[:], 0.0)
    psAB = psA.tile([P, 2, EC, 256], F32)   # [e-part, side(0=U,1=W), co, y/x]

    # ---- fp16 casts ----
    U16 = consts.tile([P, DC, E], F16)
    W16 = consts.tile([P, EC, E], F16)
    enc16 = consts.tile([P, XC, E], F16)
    nc.vector.tensor_copy(_mk(U16, 0, [(1, 512)]), _mk(U_sb, 0, [(1, 512)]))
    nc.scalar.copy(_mk(W16, 0, [(1, 512)]), _mk(W_sb, 0, [(1, 512)]))

    V2_sb = consts.tile([P, EC], F32)
    nc.vector.tensor_scalar_mul(out=V2_sb[:], in0=V_sb[:], scalar1=0.5)

    # ---- fp32 PE transposes; evacs cast to fp16 on DVE ----
    decT16 = consts.tile([P, DC, Ty], F16)
    encT16 = consts.tile([P, EC, Tx], F16)
    for i in range(YC):
        for j in range(DC):
            pt = pieces.tile([P, 512], F32, tag="piece", name=f"ptD{i}{j}")
            nc.tensor.transpose(out=pt[:, :P],
                                in_=dec_sb[:, i, j * P:(j + 1) * P],
                                identity=ident[:])
            nc.vector.tensor_copy(decT16[:, j, i * P:(i + 1) * P], pt[:, :P])
    for i in range(XC):
        for j in range(EC):
            pt = pieces.tile([P, 512], F32, tag="piece", name=f"ptE{i}{j}")
            nc.tensor.transpose(out=pt[:, :P],
                                in_=enc_sb[:, i, j * P:(j + 1) * P],
                                identity=ident[:])
            nc.vector.tensor_copy(encT16[:, j, i * P:(i + 1) * P], pt[:, :P])

    # ---- UhT / WsT ----
    for co in range(EC):
        for ci in range(DC):
            nc.tensor.matmul(
                out=psAB[:, 0, co, :],
                lhsT=U16[:, ci, co * P:(co + 1) * P],
                rhs=decT16[:, ci, :],
                start=(ci == 0), stop=(ci == DC - 1))
    for co in range(EC):
        for ci in range(EC):
            nc.tensor.matmul(
                out=psAB[:, 1, co, :],
                lhsT=W16[:, ci, co * P:(co + 1) * P],
                rhs=encT16[:, ci, :],
                start=(ci == 0), stop=(ci == EC - 1))

    # ---- factor tiles ----
    fam1 = consts.tile([P, 3072], F16)
    famA = consts.tile([P, 4096], F16)   # [fam2' | fam3]
    famB = consts.tile([P, 4096], F16)   # [fam4' | fam5]
    famC = consts.tile([P, 4096], F16)   # [fam6' | fam7]
    t2sq = consts.tile([P, 1024], F16)
    t2c2 = consts.tile([P, 1024], F16)
    m3m = consts.tile([P, 2048], F16)
    famUs = [None] * (M_MODES + 1)
    for m in range(1, M_MODES + 1):
        famUs[m] = consts.tile([P, 1024], F16, name=f"famUs{m}")

    # ---- merged trig bases over [UhT|WsT] ----
    ps_in = _mk(psAB, 0, [(512, 2), (1, 512)])
    nc.scalar.activation(out=_mk(fam1, 0, [(2048, 2), (1, 512)]),
                         in_=ps_in, func=SIN, scale=OMEGA)
    nc.scalar.activation(out=_mk(fam1, 1024, [(1536, 2), (1, 512)]),
                         in_=ps_in, func=SIN, scale=-OMEGA,
                         bias=halfpi_sb[:])
    nc.scalar.activation(out=_mk(t2sq, 0, [(1, 512)]),
                         in_=_mk(fam1, 1024, [(1, 512)]), func=SQUARE)
    nc.scalar.activation(out=_mk(t2sq, 512, [(1, 512)]),
                         in_=_mk(fam1, 2560, [(1, 512)]), func=SQUARE)

    def famUs_act(m, src, base, gamma):
        nc.scalar.activation(
            out=_mk(famUs[m], 0, [(512, 2), (1, 512)]),
            in_=_mk(src, base, [(1024, 2), (1, 512)]),
            func=COPY, scale=float(gamma))

    def famUs_dve(m, src, base, gamma):
        nc.vector.tensor_scalar_mul(
            out=_mk(famUs[m], 0, [(512, 2), (1, 512)]),
            in0=_mk(src, base, [(1024, 2), (1, 512)]),
            scalar1=float(gamma))

    # ---- DVE factor chain ----
    for ec in range(EC):
        nc.vector.tensor_scalar_mul(
            out=_mk(fam1, 512 + ec * 256, [(1, 256)]),
            in0=_mk(fam1, 2048 + ec * 256, [(1, 256)]),
            scalar1=V_sb[:, ec:ec + 1])
    for ec in range(EC):
        nc.vector.tensor_scalar_mul(
            out=_mk(fam1, 1536 + ec * 256, [(1, 256)]),
            in0=_mk(fam1, 2560 + ec * 256, [(1, 256)]),
            scalar1=V_sb[:, ec:ec + 1])
    famUs_act(1, fam1, 0, COEF[0])
    nc.vector.tensor_tensor(
        out=_mk(famA, 0, [(512, 2), (1, 512)]),
        in0=_mk(fam1, 0, [(2048, 2), (1, 512)]),
        in1=_mk(fam1, 1024, [(1536, 2), (1, 512)]), op=MULT)
    for ec in range(EC):
        nc.vector.tensor_scalar_mul(
            out=_mk(famA, 512 + ec * 256, [(1, 256)]),
            in0=_mk(famA, 512 + ec * 256, [(1, 256)]),
            scalar1=V_sb[:, ec:ec + 1])
    nc.vector.tensor_scalar(out=t2c2[:], in0=t2sq[:], scalar1=4.0,
                            scalar2=2.0, op0=MULT, op1=SUB)
    nc.vector.tensor_scalar(out=_mk(m3m, 0, [(1, 1024)]), in0=t2sq[:],
                            scalar1=4.0, scalar2=1.0, op0=MULT, op1=SUB)
    nc.vector.tensor_scalar(out=_mk(m3m, 1024, [(1, 1024)]), in0=t2sq[:],
                            scalar1=4.0, scalar2=3.0, op0=MULT, op1=SUB)
    nc.vector.tensor_scalar(out=_mk(famA, 1024, [(1, 512)]),
                            in0=_mk(t2sq, 0, [(1, 512)]),
                            scalar1=0.5, scalar2=None, op0=SUB)
    for ec in range(EC):
        nc.vector.tensor_scalar(
            out=_mk(famA, 1536 + ec * 256, [(1, 256)]),
            in0=_mk(t2sq, 512 + ec * 256, [(1, 256)]),
            scalar1=0.5, scalar2=V_sb[:, ec:ec + 1], op0=SUB, op1=MULT)
    famUs_act(2, famA, 0, 4.0 * COEF[1])
    nc.vector.tensor_tensor(out=_mk(famA, 2048, [(1, 2048)]),
                            in0=_mk(fam1, 0, [(1, 2048)]),
                            in1=_mk(m3m, 0, [(1, 2048)]), op=MULT)
    famUs_act(3, famA, 2048, COEF[2])
    nc.scalar.activation(out=warm_sb[:], in_=_mk(famUs[3], 0, [(1, 1)]),
                         func=EXP)
    t2c2_b4 = _mk(t2c2, 0, [(0, 2), (0, 2), (1, 1024)])
    sh4096 = [(2048, 2), (1024, 2), (1, 1024)]
    nc.vector.tensor_tensor(out=_mk(famB, 0, sh4096),
                            in0=_mk(famA, 0, sh4096),
                            in1=t2c2_b4, op=MULT)
    nc.vector.tensor_scalar(out=_mk(famB, 1024, [(1, 512)]),
                            in0=_mk(famB, 1024, [(1, 512)]),
                            scalar1=0.5, scalar2=None, op0=SUB)
    for ec in range(EC):
        nc.vector.tensor_scalar(
            out=_mk(famB, 1536 + ec * 256, [(1, 256)]),
            in0=_mk(famB, 1536 + ec * 256, [(1, 256)]),
            scalar1=V2_sb[:, ec:ec + 1], scalar2=None, op0=SUB)
    famUs_act(4, famB, 0, 4.0 * COEF[3])
    nc.scalar.copy(_mk(enc16, 0, [(1, 512)]),
                   _mk(enc_sb, 0, [(1, 512)]))   # ACT mid-chain slack
    nc.vector.tensor_tensor(out=_mk(famB, 2048, [(1, 2048)]),
                            in0=_mk(famB, 2048, [(1, 2048)]),
                            in1=_mk(fam1, 0, [(1, 2048)]), op=SUB)
    famUs_act(5, famB, 2048, COEF[4])
    if M_MODES >= 7:
        nc.vector.tensor_tensor(out=_mk(famC, 0, sh4096),
                                in0=_mk(famB, 0, sh4096),
                                in1=t2c2_b4, op=MULT)
        nc.vector.tensor_tensor(out=_mk(famC, 2048, [(1, 2048)]),
                                in0=_mk(famC, 2048, [(1, 2048)]),
                                in1=_mk(famA, 2048, [(1, 2048)]), op=SUB)
        with tc.high_priority():
            famUs_dve(7, famC, 2048, COEF[6])
        nc.vector.tensor_tensor(out=_mk(famC, 0, [(1, 2048)]),
                                in0=_mk(famC, 0, [(1, 2048)]),
                                in1=_mk(famA, 0, [(1, 2048)]), op=SUB)
        with tc.high_priority():
            famUs_dve(6, famC, 0, 4.0 * COEF[5])
    else:
        sh2048 = [(1024, 2), (1, 1024)]
        nc.vector.tensor_tensor(out=_mk(famC, 0, sh2048),
                                in0=_mk(famB, 0, sh2048),
                                in1=_mk(t2c2, 0, [(0, 2), (1, 1024)]),
                                op=MULT)
        nc.vector.tensor_tensor(out=_mk(famC, 0, [(1, 2048)]),
                                in0=_mk(famC, 0, [(1, 2048)]),
                                in1=_mk(famA, 0, [(1, 2048)]), op=SUB)
        with tc.high_priority():
            famUs_dve(6, famC, 0, 4.0 * COEF[5])

    # ---- logit matmuls ----
    fam_base = {1: (fam1, 0), 2: (famA, 0), 3: (famA, 2048),
                4: (famB, 0), 5: (famB, 2048), 6: (famC, 0),
                7: (famC, 2048)}

    def emit_mode(m, xhs):
        t, base = fam_base[m]
        for xh in xhs:
            for f in range(2):
                for ec in range(EC):
                    nc.tensor.matmul(
                        out=logit_ps[:, xh, :],
                        lhsT=_mk(t, base + f * 1024 + 512 + ec * 256 + xh * P,
                                 [(1, P)]),
                        rhs=_mk(famUs[m], (1 - f) * 512 + ec * 256,
                                [(1, 256)]),
                        start=False, stop=False, skip_group_check=True)

    last_modes = [M_MODES, M_MODES - 1]
    for m in range(1, M_MODES - 1):
        emit_mode(m, range(XC))
    # PE keepalives: famB sin half after its mult, then famC right after
    # its mult (holds the clock through the fam6/fam7 subtract window; the
    # WAR on famC resolves long before the subs need the buffer)
    for r, src in enumerate((_mk(famB, 512, [(1, P)]),
                             _mk(famB, 768, [(1, P)]),
                             _mk(famUs[M_MODES], 0, [(1, P)]),
                             _mk(famUs[M_MODES], 512, [(1, P)]))):
        ka = pieces.tile([P, 512], F32, tag="piece", name=f"ka{r}")
        nc.tensor.matmul(out=ka[:, :256], lhsT=src,
                         rhs=pe_warm[:], start=True, stop=True,
                         skip_group_check=True)
    expT = consts.tile([P, XC, Ty], F16)
    for m in last_modes:
        emit_mode(m, [0])
    nc.scalar.activation(out=expT[:, 0, :], in_=logit_ps[:, 0, :], func=EXP)
    for m in last_modes:
        emit_mode(m, [1])
    nc.scalar.activation(out=expT[:, 1, :], in_=logit_ps[:, 1, :], func=EXP)

    # ---- softmax epilogue ----
    recip_sb = consts.tile([P, YC], F32)
    alpha = [[consts.tile([P, P], F32, name=f"al{yh}{xh}") for xh in range(XC)]
             for yh in range(YC)]
    c_sb = [consts.tile([P, E], F32, name=f"c{yh}") for yh in range(YC)]
    # den/recip/context for both halves FIRST (keeps the pieces-pool
    # rotation acyclic: the pa transposes below alias these buffers)
    for yh in range(YC):
        den = pieces.tile([P, 512], F32, tag="piece", name=f"den{yh}")
        for xh in range(XC):
            nc.tensor.matmul(out=den[:, :1],
                             lhsT=expT[:, xh, yh * P:(yh + 1) * P],
                             rhs=ones16[:],
                             start=(xh == 0), stop=(xh == XC - 1))
        nc.vector.reciprocal(recip_sb[:, yh:yh + 1], den[:, :1])
        cps = pieces.tile([P, 512], F32, tag="piece", name=f"cps{yh}")
        for xh in range(XC):
            nc.tensor.matmul(out=cps[:, :E],
                             lhsT=expT[:, xh, yh * P:(yh + 1) * P],
                             rhs=enc16[:, xh, :],
                             start=(xh == 0), stop=(xh == XC - 1))
        nc.vector.tensor_scalar_mul(out=c_sb[yh][:], in0=cps[:, :E],
                                    scalar1=recip_sb[:, yh:yh + 1])
        nc.scalar.dma_start(out=c_d[yh * P:(yh + 1) * P, :], in_=c_sb[yh][:])
    for yh in range(YC):
        for xh in range(XC):
            pa = pieces.tile([P, 512], F16, tag="piece", name=f"pa{yh}{xh}")
            nc.tensor.transpose(out=pa[:, :P],
                                in_=expT[:, xh, yh * P:(yh + 1) * P],
                                identity=ident16[:])
            # the LAST scale must not be on ACT: the e-DMA issues on the
            # scalar ring block its sequencer ~1us (recurring trace gap)
            if xh == 0 or yh == YC - 1:
                nc.vector.tensor_scalar_mul(
                    out=alpha[yh][xh][:], in0=pa[:, :P],
                    scalar1=recip_sb[:, yh:yh + 1])
            else:
                nc.scalar.activation(out=alpha[yh][xh][:],
                                     in_=pa[:, :P], func=IDENT,
                                     scale=recip_sb[:, yh:yh + 1])
            eng = nc.sync if xh == 0 else nc.scalar
            eng.dma_start(
                out=e_d[yh * P:(yh + 1) * P, xh * P:(xh + 1) * P],
                in_=alpha[yh][xh][:])


def _build():
    nc = bacc.Bacc("TRN2", target_bir_lowering=False, debug=False,
                   num_devices=NCORES)
    enc_d = nc.dram_tensor("enc", [Tx, E], F32, kind="ExternalInput").ap()
    dec_d = nc.dram_tensor("dec", [Ty, D], F32, kind="ExternalInput").ap()
    W_d = nc.dram_tensor("W", [E, E], F32, kind="ExternalInput").ap()
    U_d = nc.dram_tensor("U", [D, E], F32, kind="ExternalInput").ap()
    V_d = nc.dram_tensor("V", [E, 1], F32, kind="ExternalInput").ap()
    c_d = nc.dram_tensor("c_out", [Ty, E], F32, kind="ExternalOutput").ap()
    e_d = nc.dram_tensor("e_out", [Ty, Tx], F32, kind="ExternalOutput").ap()

    with tile.TileContext(nc) as tc:
        with ExitStack() as ctx:
            _build_body(tc, ctx, enc_d, dec_d, W_d, U_d, V_d, c_d, e_d)
    nc.compile()
    return nc


def _get_nc():
    global _NC
    if _NC is None:
        _NC = _build()
    return _NC


def kernel(encoder_out_seq, decoder_out_seq, W_a, U_a, V_a):
    enc = np.ascontiguousarray(np.asarray(encoder_out_seq, dtype=np.float32))
    dec = np.ascontiguousarray(np.asarray(decoder_out_seq, dtype=np.float32))
    W = np.ascontiguousarray(np.asarray(W_a, dtype=np.float32))
    U = np.ascontiguousarray(np.asarray(U_a, dtype=np.float32))
    V = np.ascontiguousarray(np.asarray(V_a, dtype=np.float32))

    nc = _get_nc()
    in_maps = [
        {"enc": enc[i], "dec": dec[i], "W": W, "U": U, "V": V}
        for i in range(NCORES)
    ]
    res = run_bass_kernel_spmd(nc, in_maps, list(range(NCORES)))
    global LAST_RESULTS
    LAST_RESULTS = res
    c = np.stack([res.results[i]["c_out"] for i in range(NCORES)])
    e = np.stack([res.results[i]["e_out"] for i in range(NCORES)])
    return c, e
